# revision 12
# baseline (speedup 1.0000x reference)
"""Trainium2 Bass kernel for nn_Attention_85710367359290 (sparse branch-routed attention).

Semantics (validated vs reference):
  q = rope(a @ Wq) per branch (NB=4), k = rope(x @ Wk), v = a @ Wv per branch
  att[b,n,t,s] = q.k/sqrt(C);  m = max_n att;  p = exp(m) (no max-sub, |att|<~8)
  routing: combined_n = p * (att_n >= m) on causal positions
  y = sum_n combined_n @ v_n;  Z = sum_s p;  out = (y/Z) @ Wo

Two-phase SPMD over 8 cores (no collectives; host reshuffles between phases):
  Phase A: q/k/v projections + ropes, fully distributed - core i owns a 512-row
           T-slice of batch i//4. DMA-bound (~55us): inputs stream first, then
           outputs, with all compute under the DMA shadow.
  Phase B: attention - core (b,j) owns four 128-row t-blocks {j, 7-j, 8+j, 15-j}
           (causally balanced: s-chunk needs are {j+1, 8-j, 9+j, 16-j}, padded
           uniformly to NEED=(4,8,12,16) = 40 trips vs exact 34). Ring schedule:
           trip (g, si) runs at tau = g + si so v[si]/kr[si] are each loaded
           once and shared by all segments. Routing per trip: e=exp(att) [Act]
           -> m=max_n [DVE reduce] -> ge=(e>=m) [DVE] -> pm=m*mask [DVE] ->
           cmb=ge*pm [DVE 4x bf16]; Z and PV accumulate in PSUM per segment
           (one accumulation group per 2KB PSUM bank - the hardware start flag
           zeroes the whole bank); per-segment epilogue does o_proj + 1/Z.
"""

import numpy as np
import ml_dtypes

import concourse.bass as bass
import concourse.mybir as mybir
import concourse.tile as tile
from concourse import bacc
from concourse.bass_utils import run_bass_kernel_spmd

F32 = mybir.dt.float32
F32R = mybir.dt.float32r
BF16 = mybir.dt.bfloat16
ALU = mybir.AluOpType
ACTF = mybir.ActivationFunctionType
AX = mybir.AxisListType

B, T, C, NB = 2, 2048, 512, 4
N_CORES = 8

QKD = F32R
VD = BF16
NPVD = ml_dtypes.bfloat16

NEED = [4, 8, 12, 16]          # padded s-chunk counts per segment
NTRIPS = sum(NEED)             # 40


def _blocks(j):
    return [j, 7 - j, 8 + j, 15 - j]


def _trip_schedule():
    """Static (core-independent) trip order: trip (g, si) at tau = g + si."""
    out = []
    for tau in range(19):
        for g in range(4):
            si = tau - g
            if 0 <= si < NEED[g]:
                out.append((g, si))
    assert len(out) == NTRIPS
    return out


TRIPS = _trip_schedule()

_cache = {}


def build_phase_a():
    if "a" in _cache:
        return _cache["a"]
    nc = bacc.Bacc("TRN2", target_bir_lowering=False, debug=False)

    def din(name, shape, dt):
        return nc.dram_tensor(name, shape, dt, kind="ExternalInput").ap()

    aT = din("aT", [C, 512], QKD)      # a[b].T cols of this core's T-slice
    aTb = din("aTb", [C, 512], VD)     # same, bf16 (for v)
    xT = din("xT", [C, 512], QKD)
    Wq = din("Wq", [C, NB * C], QKD)   # split-permuted
    Wk = din("Wk", [C, C], QKD)        # split-permuted, pre-scaled 1/sqrt(C)
    Wv = din("Wv", [C, NB * C], VD)
    cosA = din("cosA", [C // 2, 512], F32)
    sinA = din("sinA", [C // 2, 512], F32)
    qrA = nc.dram_tensor("qrA", [NB * C, 512], QKD, kind="ExternalOutput").ap()
    krA = nc.dram_tensor("krA", [C, 512], QKD, kind="ExternalOutput").ap()
    vA = nc.dram_tensor("vA", [512, NB * C], VD, kind="ExternalOutput").ap()

    with tile.TileContext(nc) as tc:
        with (
            tc.tile_pool(name="pa", bufs=1) as pa,
            tc.tile_pool(name="pat", bufs=4) as pat,
            tc.tile_pool(name="pap", bufs=6, space="PSUM") as pps,
        ):
            aTt = [pa.tile([128, 512], QKD, tag=f"aT{i}", name=f"aT{i}") for i in range(4)]
            aTbt = [pa.tile([128, 512], VD, tag=f"aTb{i}", name=f"aTb{i}") for i in range(4)]
            xTt = [pa.tile([128, 512], QKD, tag=f"xT{i}", name=f"xT{i}") for i in range(4)]
            WqT = [pa.tile([128, NB * C], QKD, tag=f"Wq{i}", name=f"Wq{i}") for i in range(4)]
            WkT = [pa.tile([128, C], QKD, tag=f"Wk{i}", name=f"Wk{i}") for i in range(4)]
            WvT = [pa.tile([128, NB * C], VD, tag=f"Wv{i}", name=f"Wv{i}") for i in range(4)]
            cst = [pa.tile([128, 512], F32, tag=f"cs{i}", name=f"cs{i}") for i in range(2)]
            snt = [pa.tile([128, 512], F32, tag=f"sn{i}", name=f"sn{i}") for i in range(2)]

            # all input DMAs first (DMA queue is in-order and head-blocking;
            # output DMAs are emitted last, in compute-readiness order)
            for i in range(4):
                nc.sync.dma_start(out=xTt[i], in_=xT[i * 128:(i + 1) * 128, :])
                nc.sync.dma_start(out=WkT[i], in_=Wk[i * 128:(i + 1) * 128, :])
            for i in range(2):
                nc.sync.dma_start(out=cst[i], in_=cosA[i * 128:(i + 1) * 128, :])
                nc.sync.dma_start(out=snt[i], in_=sinA[i * 128:(i + 1) * 128, :])
            for i in range(4):
                nc.sync.dma_start(out=aTt[i], in_=aT[i * 128:(i + 1) * 128, :])
            for ncc in range(4):           # Wq by branch-column blocks
                for Kc in range(4):
                    nc.sync.dma_start(
                        out=WqT[Kc][:, ncc * 512:(ncc + 1) * 512],
                        in_=Wq[Kc * 128:(Kc + 1) * 128, ncc * 512:(ncc + 1) * 512])
            for i in range(4):
                nc.sync.dma_start(out=aTbt[i], in_=aTb[i * 128:(i + 1) * 128, :])
            for i in range(4):
                nc.sync.dma_start(out=WvT[i], in_=Wv[i * 128:(i + 1) * 128, :])

            # ---- k proj + rope ----
            kpre = [pa.tile([128, 512], F32, tag=f"kpre{i}", name=f"kpre{i}") for i in range(4)]
            for m in range(4):
                ps = pps.tile([128, 512], F32, tag="pps", name="pps")
                for Kc in range(4):
                    nc.tensor.matmul(ps, WkT[Kc][:, m * 128:(m + 1) * 128], xTt[Kc],
                                     start=(Kc == 0), stop=(Kc == 3))
                nc.scalar.copy(out=kpre[m], in_=ps)
            krt = []
            for h in range(2):
                t1 = pat.tile([128, 512], F32, tag="t1", name="t1")
                t2 = pat.tile([128, 512], F32, tag="t2", name="t2")
                kr = pa.tile([128, 512], QKD, tag=f"krr{h}", name=f"krr{h}")
                nc.vector.tensor_mul(t1, kpre[h], cst[h])
                nc.vector.tensor_mul(t2, kpre[2 + h], snt[h])
                nc.vector.tensor_sub(kr, t1, t2)
                t3 = pat.tile([128, 512], F32, tag="t3", name="t3")
                t4 = pat.tile([128, 512], F32, tag="t4", name="t4")
                kr2 = pa.tile([128, 512], QKD, tag=f"krr{2 + h}", name=f"krr{2 + h}")
                nc.vector.tensor_mul(t3, kpre[h], snt[h])
                nc.vector.tensor_mul(t4, kpre[2 + h], cst[h])
                nc.vector.tensor_add(kr2, t3, t4)
                krt.append((h, kr, kr2))

            # ---- q proj + rope (per branch, streams behind Wq chunks) ----
            qpre = [pa.tile([128, 512], F32, tag=f"qpre{i}", name=f"qpre{i}") for i in range(4)]
            qrt = []
            for n in range(NB):
                for m in range(4):
                    ps = pps.tile([128, 512], F32, tag="pps", name="pps")
                    for Kc in range(4):
                        nc.tensor.matmul(
                            ps, WqT[Kc][:, (4 * n + m) * 128:(4 * n + m + 1) * 128],
                            aTt[Kc], start=(Kc == 0), stop=(Kc == 3))
                    nc.scalar.copy(out=qpre[m], in_=ps)
                for h in range(2):
                    t1 = pat.tile([128, 512], F32, tag="qt1", name="qt1")
                    t2 = pat.tile([128, 512], F32, tag="qt2", name="qt2")
                    qr = pa.tile([128, 512], QKD, tag=f"qrr{4 * n + h}", name=f"qrr{4 * n + h}")
                    nc.vector.tensor_mul(t1, qpre[h], cst[h])
                    nc.vector.tensor_mul(t2, qpre[2 + h], snt[h])
                    nc.vector.tensor_sub(qr, t1, t2)
                    qrt.append((4 * n + h, qr))
                    t3 = pat.tile([128, 512], F32, tag="qt3", name="qt3")
                    t4 = pat.tile([128, 512], F32, tag="qt4", name="qt4")
                    qr2 = pa.tile([128, 512], QKD, tag=f"qrr{4 * n + 2 + h}",
                                  name=f"qrr{4 * n + 2 + h}")
                    nc.vector.tensor_mul(t3, qpre[h], snt[h])
                    nc.vector.tensor_mul(t4, qpre[2 + h], cst[h])
                    nc.vector.tensor_add(qr2, t3, t4)
                    qrt.append((4 * n + 2 + h, qr2))

            # ---- output DMAs in readiness order: kr, qr, then v inline ----
            for h, kr, kr2 in krt:
                nc.sync.dma_start(out=krA[h * 128:(h + 1) * 128, :], in_=kr)
                nc.sync.dma_start(out=krA[(2 + h) * 128:(3 + h) * 128, :], in_=kr2)
            for row, qr in qrt:
                nc.sync.dma_start(out=qrA[row * 128:(row + 1) * 128, :], in_=qr)

            # ---- v proj (last; writes trail the DMA queue) ----
            for sc in range(4):
                for nb in range(4):
                    ps = pps.tile([128, 512], F32, tag="pps", name="pps")
                    for Kc in range(4):
                        nc.tensor.matmul(ps, aTbt[Kc][:, sc * 128:(sc + 1) * 128],
                                         WvT[Kc][:, nb * 512:(nb + 1) * 512],
                                         start=(Kc == 0), stop=(Kc == 3))
                    vs = pat.tile([128, 512], VD, tag="vs", name="vs")
                    nc.scalar.copy(out=vs, in_=ps)
                    nc.sync.dma_start(
                        out=vA[sc * 128:(sc + 1) * 128, nb * 512:(nb + 1) * 512], in_=vs)
    nc.compile()
    _cache["a"] = nc
    return nc


def build_phase_b():
    if "b" in _cache:
        return _cache["b"]
    nc = bacc.Bacc("TRN2", target_bir_lowering=False, debug=False)

    def din(name, shape, dt):
        return nc.dram_tensor(name, shape, dt, kind="ExternalInput").ap()

    # QT layout: 4 Kc-tiles [128, 2048]; col = g*512 + n*128 + t  (n = branch)
    qpB = din("qp", [4 * 128, 2048], QKD)
    krB = din("krB", [C, T], QKD)          # [c', s]
    vB = din("vB", [T, NB * C], VD)        # [s, n*512+c]
    WoD = din("Wo", [C, C], VD)
    mskD = din("msk", [128, NTRIPS * 128], BF16)   # [s, trip*128+t]
    out = nc.dram_tensor("o", [512, C], F32, kind="ExternalOutput").ap()

    first_use = {}
    for k, (g, si) in enumerate(TRIPS):
        first_use.setdefault(si, k)
    v_emit = {}
    for si, k in first_use.items():
        v_emit.setdefault(max(0, k - 2), []).append(si)

    with tile.TileContext(nc) as tc:
        with (
            tc.tile_pool(name="pp", bufs=1) as pp,
            tc.tile_pool(name="pv", bufs=5) as pv,
            tc.tile_pool(name="pe", bufs=3) as pe,
            tc.tile_pool(name="pr", bufs=3) as pr,
            tc.tile_pool(name="pw", bufs=2) as pw,
            tc.tile_pool(name="patt", bufs=3, space="PSUM") as patt,
            tc.tile_pool(name="pacc", bufs=1, space="PSUM") as pacc,
        ):
            QT = [pp.tile([128, 2048], QKD, tag=f"QT{i}", name=f"QT{i}") for i in range(4)]
            krT = [pp.tile([128, 2048], QKD, tag=f"krT{i}", name=f"krT{i}") for i in range(4)]
            WoT = [pp.tile([128, C], VD, tag=f"Wo{i}", name=f"Wo{i}") for i in range(4)]
            mskT = pp.tile([128, NTRIPS * 128], BF16, tag="mskT", name="mskT")
            ones = pp.tile([128, 1], VD, tag="ones", name="ones")
            nc.vector.memset(ones, 1.0)

            yT = [pacc.tile([128, 512], F32, tag=f"yT{i}", name=f"yT{i}") for i in range(4)]
            # full-bank tile (cols 0..3 used): PSUM start marks a whole 2KB
            # zero-region, so Zp owns its bank and uses ONE accum group
            Zp = pacc.tile([128, 512], F32, tag="Zp", name="Zp")

            def ld_kr(cb, w=512):
                for Kc in range(4):
                    nc.sync.dma_start(out=krT[Kc][:, cb * 128:cb * 128 + w],
                                      in_=krB[Kc * 128:(Kc + 1) * 128, cb * 128:cb * 128 + w])

            def ld_qp(g):
                for Kc in range(4):
                    nc.sync.dma_start(out=QT[Kc][:, g * 512:(g + 1) * 512],
                                      in_=qpB[Kc * 128:(Kc + 1) * 128, g * 512:(g + 1) * 512])

            ld_kr(0, 128)                      # kr s-chunk 0
            ld_qp(0)
            nc.sync.dma_start(out=mskT[:, :8 * 128], in_=mskD[:, :8 * 128])
            ld_kr(1, 128)
            deferred = [lambda: (ld_qp(1), ld_kr(2, 256)),
                        lambda: ld_qp(2),
                        lambda: [nc.sync.dma_start(out=WoT[i], in_=WoD[i * 128:(i + 1) * 128, :])
                                 for i in range(4)],
                        lambda: ld_qp(3),
                        lambda: nc.sync.dma_start(out=mskT[:, 8 * 128:], in_=mskD[:, 8 * 128:]),
                        lambda: ld_kr(4, 512),
                        lambda: ld_kr(8, 512),
                        lambda: ld_kr(12, 512)]

            vt = {}
            for k, (g, si) in enumerate(TRIPS):
                for vsi in v_emit.get(k, []):
                    vt[vsi] = pv.tile([128, NB * C], VD, tag="vt", name=f"v{vsi}")
                    nc.sync.dma_start(out=vt[vsi], in_=vB[vsi * 128:(vsi + 1) * 128, :])
                if deferred:
                    deferred.pop(0)()

                att = patt.tile([128, 512], F32, tag="att", name="att")
                # one accum group per bank: start/stop only on first/last matmul
                for np_ in range(2):
                    for Kc in range(4):
                        nc.tensor.matmul(
                            att[:, np_ * 256:(np_ + 1) * 256],
                            krT[Kc][:, si * 128:(si + 1) * 128],
                            QT[Kc][:, g * 512 + np_ * 256:g * 512 + (np_ + 1) * 256],
                            start=(np_ == 0 and Kc == 0), stop=(np_ == 1 and Kc == 3))
                e = pe.tile([128, 512], F32, tag="e", name="e")
                nc.scalar.activation(out=e, in_=att, func=ACTF.Exp)
                m = pr.tile([128, 128], F32, tag="m", name="m")
                nc.vector.tensor_reduce(m, e.rearrange("p (n t) -> p t n", n=4),
                                        AX.X, ALU.max)
                ge = pr.tile([128, 512], BF16, tag="ge", name="ge")
                mb = m.unsqueeze(1).broadcast_to([128, 4, 128])
                nc.vector.tensor_tensor(out=ge.rearrange("p (n t) -> p n t", n=4),
                                        in0=e.rearrange("p (n t) -> p n t", n=4),
                                        in1=mb, op=ALU.is_ge)
                pm = pr.tile([128, 128], BF16, tag="pm", name="pm")
                nc.vector.tensor_mul(pm, m, mskT[:, k * 128:(k + 1) * 128])
                cmb = pr.tile([128, 512], BF16, tag="cmb", name="cmb")
                pmb = pm.unsqueeze(1).broadcast_to([128, 4, 128])
                nc.vector.tensor_mul(cmb.rearrange("p (n t) -> p n t", n=4),
                                     ge.rearrange("p (n t) -> p n t", n=4), pmb)
                # Z[t] += sum_s pm; single accum group over ALL trips (the four
                # columns share one 2KB zero-region)
                nc.tensor.matmul(Zp[:, g:g + 1], pm, ones,
                                 start=(k == 0), stop=(k == NTRIPS - 1))
                # PV: yT[g][c-tile Mc] += v_n^T chunk x cmb_n; one group per bank
                for n in range(4):
                    for Mc in range(4):
                        nc.tensor.matmul(
                            yT[g][:, Mc * 128:(Mc + 1) * 128],
                            vt[si][:, n * 512 + Mc * 128:n * 512 + (Mc + 1) * 128],
                            cmb[:, n * 128:(n + 1) * 128],
                            start=(si == 0 and n == 0 and Mc == 0),
                            stop=(si == NEED[g] - 1 and n == 3 and Mc == 3))
                if si == NEED[g] - 1:
                    yb = pw.tile([128, 512], VD, tag="yb", name="yb")
                    nc.scalar.copy(out=yb, in_=yT[g])
                    zr = pw.tile([128, 1], F32, tag="zr", name="zr")
                    nc.vector.reciprocal(zr, Zp[:, g:g + 1])
                    # o_proj PSUM comes from the att pool (frees a bank for the
                    # deeper att rotation)
                    ops = patt.tile([128, 512], F32, tag="att", name="ops")
                    for Mc in range(4):
                        nc.tensor.matmul(ops, yb[:, Mc * 128:(Mc + 1) * 128], WoT[Mc],
                                         start=(Mc == 0), stop=(Mc == 3))
                    osb = pw.tile([128, 512], F32, tag="osb", name="osb")
                    nc.scalar.mul(osb, ops, zr)
                    nc.sync.dma_start(out=out[g * 128:(g + 1) * 128, :], in_=osb)
            assert not deferred
    nc.compile()
    _cache["b"] = nc
    return nc


def _masks(j):
    """Per-trip causal masks [s, trip*128+t], bf16, in TRIPS order."""
    mm = _blocks(j)
    msk = np.zeros((128, NTRIPS * 128), np.float32)
    ss = np.arange(128)[:, None]
    tt = np.arange(128)[None, :]
    for k, (g, si) in enumerate(TRIPS):
        msk[:, k * 128:(k + 1) * 128] = (128 * mm[g] + tt) >= (128 * si + ss)
    return msk.astype(ml_dtypes.bfloat16)


def kernel(a, x, Wq, Wk, Wv, Wo, cos, sin, _trace=False):
    a = np.asarray(a, np.float32)
    x = np.asarray(x, np.float32)
    Wq = np.asarray(Wq, np.float32)
    Wk = np.asarray(Wk, np.float32)
    Wv = np.asarray(Wv, np.float32)
    Wo = np.asarray(Wo, np.float32)
    cos = np.asarray(cos, np.float32)
    sin = np.asarray(sin, np.float32)

    split_idx = np.r_[0:C:2, 1:C:2]
    Wq_p = np.ascontiguousarray(Wq.reshape(C, NB, C)[:, :, split_idx].reshape(C, NB * C))
    Wk_p = np.ascontiguousarray(Wk[:, split_idx] * np.float32(1.0 / np.sqrt(C)))
    Wv_b = Wv.astype(NPVD)
    Wo_b = Wo.astype(NPVD)
    cosTf = np.ascontiguousarray(cos[:T].T)
    sinTf = np.ascontiguousarray(sin[:T].T)

    # ---- phase A ----
    nca = build_phase_a()
    in_a = []
    for core in range(N_CORES):
        b, s4 = divmod(core, 4)
        rows = slice(512 * s4, 512 * (s4 + 1))
        aTs = np.ascontiguousarray(a[b].T[:, rows])
        in_a.append({
            "aT": aTs,
            "aTb": aTs.astype(NPVD),
            "xT": np.ascontiguousarray(x[b].T[:, rows]),
            "Wq": Wq_p, "Wk": Wk_p, "Wv": Wv_b,
            "cosA": np.ascontiguousarray(cosTf[:, rows]),
            "sinA": np.ascontiguousarray(sinTf[:, rows]),
        })
    res_a = run_bass_kernel_spmd(nca, in_a, list(range(N_CORES)))

    qr_full = [np.concatenate([res_a.results[b * 4 + s]["qrA"] for s in range(4)], axis=1)
               for b in range(B)]   # [2048, 2048]
    kr_full = [np.concatenate([res_a.results[b * 4 + s]["krA"] for s in range(4)], axis=1)
               for b in range(B)]   # [512, 2048]
    v_full = [np.concatenate([res_a.results[b * 4 + s]["vA"] for s in range(4)], axis=0)
              for b in range(B)]    # [2048, 2048] bf16

    # ---- phase B ----
    ncb = build_phase_b()
    in_b = []
    for core in range(N_CORES):
        b, j = divmod(core, 4)
        mm = _blocks(j)
        qpk = np.empty((4 * 128, 2048), np.float32)
        for Kc in range(4):
            for g in range(4):
                tc_ = slice(128 * mm[g], 128 * (mm[g] + 1))
                for n in range(4):
                    qpk[Kc * 128:(Kc + 1) * 128,
                        g * 512 + n * 128:g * 512 + (n + 1) * 128] = \
                        qr_full[b][(4 * n + Kc) * 128:(4 * n + Kc + 1) * 128, tc_]
        in_b.append({
            "qp": qpk,
            "krB": kr_full[b],
            "vB": v_full[b],
            "Wo": Wo_b,
            "msk": _masks(j),
        })
    res_b = run_bass_kernel_spmd(ncb, in_b, list(range(N_CORES)))

    outf = np.zeros((B, T, C), np.float32)
    for core in range(N_CORES):
        b, j = divmod(core, 4)
        mm = _blocks(j)
        o = res_b.results[core]["o"]
        for g in range(4):
            outf[b, 128 * mm[g]:128 * (mm[g] + 1)] = o[g * 128:(g + 1) * 128]
    if _trace:
        return outf, (res_a, res_b)
    return outf
